# revision 3
# baseline (speedup 1.0000x reference)
"""BlockDiagonalChannelAttention Trainium2 kernel (8 NeuronCores).

Reference computation (per non-overlapping 7x7 patch n of 4096, C=128, P=49):
    G   = flat @ flat^T                      (C,C) gram
    Sc  = softmax(G, axis=-1)
    cov = (flat-mu)(flat-mu)^T / P = G/P - mu mu^T
    Ec  = (Sc + 0.5*cov) @ flat
    fold Ec back, attention = beta*Ec_map + x, out = x*attention

Sharding: 128 row-strips (b, hh) of x, 16 strips per core, 32 patches/strip.

Device computes per patch:
    scp = Sc   (exact: exp(G - diag) row-normalized; the diag shift is a
                valid softmax shift: G[c,d] <= max(G[c,c], G[d,d]) keeps
                exp in fp32 range)
    ap  = 98*Sc + G
    ec  = (98*Sc + G) @ flat
Host reconstructs (98*Sc + G - 49 mu mu^T = 98*(Sc + 0.5*cov)):
    G   = ap - 98*scp
    cov = G/49 - mu mu^T
    Ec  = (ec - 49 * mu (mu^T flat)) / 98
plus the fold and the final elementwise out = x*(beta*Ec_map + x).
"""

import sys

sys.path.insert(0, "/opt/trn_rl_repo")

import numpy as np

B, C, H, W = 4, 128, 224, 224
PH = PW = 7
HH = H // PH          # 32 patch rows
P = PH * PW           # 49
N_CORES = 8
N_STRIPS_TOTAL = B * HH              # 128; strip s = (b=s//32, hh=s%32)
WW = W // PW          # 32 patches per strip
SW = PH * W           # 1568 floats per strip row

_cache = {}


def _build(n_strips):
    """Build the SPMD Bass module for one core handling n_strips strips."""
    from concourse import bacc, tile, mybir, masks

    f32 = mybir.dt.float32
    AF = mybir.ActivationFunctionType
    ALU = mybir.AluOpType

    nc = bacc.Bacc("TRN2", target_bir_lowering=False, debug=False)

    xs = nc.dram_tensor("xs", [n_strips, C, SW], f32, kind="ExternalInput")
    # outputs in channel-major layout; host transposes (s,c,w,d)->(s,w,c,d)
    scp = nc.dram_tensor("scp", [n_strips, C, WW, C], f32, kind="ExternalOutput")
    ap = nc.dram_tensor("ap", [n_strips, C, WW, C], f32, kind="ExternalOutput")
    ec = nc.dram_tensor("ec", [n_strips, C, WW, P], f32, kind="ExternalOutput")

    with tile.TileContext(nc) as tc:
        with (
            tc.tile_pool(name="const", bufs=1) as constp,
            tc.tile_pool(name="strip", bufs=2) as stripp,
            tc.tile_pool(name="flats", bufs=2) as flatsp,
            tc.tile_pool(name="sq", bufs=2) as sqp,
            tc.tile_pool(name="stats", bufs=2) as statsp,
            tc.tile_pool(name="ft", bufs=3) as ftp,
            tc.tile_pool(name="fexp", bufs=6) as fexpp,
            tc.tile_pool(name="ats", bufs=3) as atsp,
            tc.tile_pool(name="oscp", bufs=2) as oscpp,
            tc.tile_pool(name="oap", bufs=2) as oapp,
            tc.tile_pool(name="oec", bufs=2) as oecp,
            tc.tile_pool(name="ps_ft", bufs=2, space="PSUM") as ps_ft,
            tc.tile_pool(name="ps_g", bufs=3, space="PSUM") as ps_g,
            tc.tile_pool(name="ps_at", bufs=2, space="PSUM") as ps_at,
            tc.tile_pool(name="ps_ec", bufs=1, space="PSUM") as ps_ec,
        ):
            ident = constp.tile([128, 128], f32)
            masks.make_identity(nc, ident[:])

            for s in range(n_strips):
                strip = stripp.tile([C, SW], f32)
                nc.sync.dma_start(strip[:], xs[s])

                # reorder (c, i, ww, j) -> (c, ww, i*7+j): patch-major columns
                flat = flatsp.tile([C, WW, P], f32)
                nc.gpsimd.tensor_copy(
                    flat[:].rearrange("c w (i j) -> c w i j", i=PH, j=PW),
                    strip[:].rearrange("c (i w j) -> c w i j", i=PH, w=WW, j=PW),
                )

                # negdiag[c, w] = -sum_p flat[c, w, p]^2  (softmax shift)
                sq = sqp.tile([C, WW, P], f32)
                nc.scalar.activation(sq[:], flat[:], AF.Square)
                negdiag = statsp.tile([C, WW], f32, tag="negdiag")
                nc.vector.tensor_reduce(
                    negdiag[:], sq[:], axis=mybir.AxisListType.X,
                    op=ALU.add, negate=True,
                )

                s_all = statsp.tile([C, WW], f32, tag="s_all")
                r_all = statsp.tile([C, WW], f32, tag="r_all")

                scp_st = oscpp.tile([C, WW * C], f32)
                ap_st = oapp.tile([C, WW * C], f32)
                ec_st = oecp.tile([C, WW * P], f32)

                for w4 in range(0, WW, 4):
                    # 4 patches share one transposed-flat PSUM bank (49 x 512)
                    ftps = ps_ft.tile([P, 4 * C], f32)
                    for k in range(4):
                        nc.tensor.transpose(
                            ftps[:, k * C:(k + 1) * C],
                            flat[:, w4 + k, :],
                            ident[:],
                        )
                    fts = ftp.tile([P, 4 * C], f32)
                    nc.any.tensor_copy(fts[:], ftps[:])

                    for w2 in (w4, w4 + 2):
                        gs = []
                        for w in (w2, w2 + 1):
                            k = w - w4
                            fT = fts[:, k * C:(k + 1) * C]
                            g = ps_g.tile([C, C], f32)
                            nc.tensor.matmul(g[:], fT, fT, start=True, stop=True)
                            # F = exp(G - diag) into SBUF, s = rowsum(F)
                            f_t = fexpp.tile([C, C], f32)
                            nc.scalar.activation(
                                f_t[:], g[:], AF.Exp,
                                bias=negdiag[:, w:w + 1],
                                accum_out=s_all[:, w:w + 1],
                            )
                            gs.append((g, f_t))

                        nc.vector.reciprocal(
                            r_all[:, w2:w2 + 2], s_all[:, w2:w2 + 2]
                        )

                        for w in (w2, w2 + 1):
                            g, f_t = gs[w - w2]
                            sc_sl = scp_st[:, w * C:(w + 1) * C]
                            ap_sl = ap_st[:, w * C:(w + 1) * C]
                            # Sc = F * (1/s)   (per-partition scalar)
                            nc.gpsimd.tensor_scalar(
                                sc_sl, f_t[:], r_all[:, w:w + 1], None,
                                op0=ALU.mult,
                            )
                            # A' = 98*Sc + G
                            nc.vector.scalar_tensor_tensor(
                                ap_sl, sc_sl, 98.0, g[:],
                                op0=ALU.mult, op1=ALU.add,
                            )
                            # A'^T via PE, copy back to SBUF
                            atps = ps_at.tile([C, C], f32)
                            nc.tensor.transpose(atps[:], ap_sl, ident[:])
                            ats = atsp.tile([C, C], f32)
                            nc.any.tensor_copy(ats[:], atps[:])
                            # Ec' = A' @ flat_w  (lhsT = A'^T)
                            if w % 8 == 0:
                                ecps = ps_ec.tile([C, 8 * P], f32)
                            wi = w % 8
                            nc.tensor.matmul(
                                ecps[:, wi * P:(wi + 1) * P],
                                ats[:], flat[:, w, :],
                                start=True, stop=True,
                            )
                            if wi == 7:
                                nc.any.tensor_copy(
                                    ec_st[:, (w - 7) * P:(w + 1) * P],
                                    ecps[:],
                                )

                nc.sync.dma_start(scp[s], scp_st[:].rearrange("c (w d) -> c w d", w=WW))
                nc.sync.dma_start(ap[s], ap_st[:].rearrange("c (w d) -> c w d", w=WW))
                nc.sync.dma_start(ec[s], ec_st[:].rearrange("c (w p) -> c w p", w=WW))

    nc.compile()
    return nc


def _get_nc(n_strips):
    if n_strips not in _cache:
        _cache[n_strips] = _build(n_strips)
    return _cache[n_strips]


def kernel(x, beta):
    from concourse.bass_utils import run_bass_kernel_spmd

    x = np.asarray(x, dtype=np.float32)
    beta = np.asarray(beta, dtype=np.float32)

    spc = N_STRIPS_TOTAL // N_CORES     # strips per core
    nc = _get_nc(spc)

    # strips: (b, hh) -> (128, C, SW)
    strips = np.ascontiguousarray(
        x.reshape(B, C, HH, SW).transpose(0, 2, 1, 3)
    ).reshape(N_STRIPS_TOTAL, C, SW)

    in_maps = [
        {"xs": np.ascontiguousarray(strips[k * spc:(k + 1) * spc])}
        for k in range(N_CORES)
    ]
    res = run_bass_kernel_spmd(nc, in_maps, list(range(N_CORES))).results

    # gather (s, c, w, ...) -> (n=s*32+w, c, ...)
    scp = np.concatenate([res[k]["scp"] for k in range(N_CORES)], axis=0)
    apo = np.concatenate([res[k]["ap"] for k in range(N_CORES)], axis=0)
    eco = np.concatenate([res[k]["ec"] for k in range(N_CORES)], axis=0)

    NT = N_STRIPS_TOTAL
    Sc = np.ascontiguousarray(scp.transpose(0, 2, 1, 3)).reshape(NT * WW, C, C)
    A_ = np.ascontiguousarray(apo.transpose(0, 2, 1, 3)).reshape(NT * WW, C, C)
    Ecp = np.ascontiguousarray(eco.transpose(0, 2, 1, 3)).reshape(NT * WW, C, P)

    # host-side flat view of x (n, c, p)
    flat = np.ascontiguousarray(
        strips.reshape(NT, C, PH, WW, PW).transpose(0, 3, 1, 2, 4)
    ).reshape(NT * WW, C, P)

    mu = flat.mean(axis=-1)                         # (n, c)
    G = A_ - 98.0 * Sc
    cov = G * (1.0 / P) - mu[:, :, None] * mu[:, None, :]
    wv = np.einsum("nc,ncp->np", mu, flat)          # mu^T @ flat
    Ec = (Ecp - 49.0 * mu[:, :, None] * wv[:, None, :]) * (1.0 / 98.0)

    # fold (n=(b,hh,ww), c, p=(i,j)) -> (b, c, hh*7+i, ww*7+j)
    Ec_map = np.ascontiguousarray(
        Ec.reshape(B, HH, WW, C, PH, PW).transpose(0, 3, 1, 4, 2, 5)
    ).reshape(B, C, H, W)

    attention = beta[0] * Ec_map + x
    out = x * attention

    M = HH * WW
    return (
        out.astype(np.float32),
        Sc.reshape(B, M, C, C),
        cov.reshape(B, M, C, C).astype(np.float32),
        Ec_map.astype(np.float32),
    )
